# revision 22
# baseline (speedup 1.0000x reference)
"""Trainium2 Bass kernel for nn_KeypointsLoss.

Math (per batch b):
    x[p,k] = trunc(kp[b,p,k,0] * (W-1)); y likewise from kp[...,1]
    g_row[p,k,h] = exp(-(h-x)^2/(2s^2)) * (vis>0);  g_col[p,k,w] = exp(-(w-y)^2/(2s^2))
    target[k] = sum_p outer(g_row, g_col)            # [H,W]
    per_sample = sum_k |pred[b,k] - target[k]|^2
    loss = sum_b per_sample / (sum(vis[b]) + 1e-6) / B

Strategy (8 cores, data-parallel over B=32 -> 4 batches/core):
  - The 1-D gaussian factor vectors (tiny: ~1MB/core) are precomputed on the
    host and land in ONE DMA; the device prologue is pure DMA - no dependent
    compute before the first matmul.
  - PE builds (target - pred) in PSUM: band-packed block-diag g_col tiles
    splat the 4-k-group targets via c=32 tile_position matmuls, then a
    (-I96) matmul accumulates -pred.
  - ScalarE (activation Square + fused row-sum) / VectorE square-reduce PSUM
    into per-batch accumulators; tiny matmul with a (1/32)-vector reduces
    partitions; host just sums the 8x4 partials.
"""

import sys
import numpy as np

sys.path.insert(0, "/opt/trn_rl_repo")

B, P, K, H, W = 32, 8, 17, 192, 192
SIGMA = 3.0
INV2S2 = 1.0 / (2.0 * SIGMA**2)
NCORES = 8
NB = B // NCORES          # batches per core
HL = 96                   # h split: [0:96) lo, [96:192) hi
KW = K * W                # 3264 free cols for pred tiles
NG = 4                    # full k-groups of 4 (k0..15); k=16 handled separately

# gtens (bf16) column layout
GD0 = 0                   # band-packed block-diag gcol: [128, NB*4*W]
GROW0 = NB * 4 * W        # grow0: cols (b, h) [128, NB*W]
GROW1 = GROW0 + NB * W    # k16 grow: [128, W] (partition 32b+p)
GCOL1 = GROW1 + W         # k16 gcol: [128, W]
GCOLS = GCOL1 + W
# ctrl (f32) column layout: negi bitcast [96, 48], ones col 48, invd col 49
CCOLS = 50

_CACHE = {}


def _build():
    import concourse.bass as bass
    import concourse.bacc as bacc
    import concourse.tile as tile
    from concourse import mybir

    f32 = mybir.dt.float32
    bf16 = mybir.dt.bfloat16
    Alu = mybir.AluOpType
    Act = mybir.ActivationFunctionType

    nc = bacc.Bacc("TRN2", target_bir_lowering=False, debug=False,
                   num_devices=NCORES)

    pred_d = nc.dram_tensor("pred", [NB, K, H, W], f32, kind="ExternalInput").ap()
    gt_d = nc.dram_tensor("gtens", [128, GCOLS], bf16, kind="ExternalInput").ap()
    ctrl_d = nc.dram_tensor("ctrl", [128, CCOLS], f32, kind="ExternalInput").ap()
    out_d = nc.dram_tensor("out", [NB, 1], f32, kind="ExternalOutput").ap()

    with tile.TileContext(nc) as tc:
        import contextlib
        with contextlib.ExitStack() as ctx:
            consts = ctx.enter_context(tc.tile_pool(name="consts", bufs=1))
            gpool = ctx.enter_context(tc.tile_pool(name="gpool", bufs=1))
            predp = ctx.enter_context(tc.tile_pool(name="pred", bufs=2))
            scrp = ctx.enter_context(tc.tile_pool(name="scr", bufs=2))
            psump = ctx.enter_context(tc.tile_pool(name="psum", bufs=2, space="PSUM"))

            # pure-DMA prologue: small control tensors first (their few
            # packets clear the SDMA engines before pred floods them)
            ctrl = consts.tile([128, CCOLS], f32, tag="ctrl")
            gt = consts.tile([128, GCOLS - GROW0], bf16, tag="gt")
            nc.sync.dma_start(out=ctrl[:], in_=ctrl_d[:])
            nc.sync.dma_start(out=gt[:], in_=gt_d[:, GROW0:GCOLS])
            # block-diag staircase tiles: zero on DVE (idle in the prologue),
            # then land each group's 32-row band straight from DRAM; full
            # 128-partition tiles keep the splat matmuls full-array (HAM
            # clock-gate ignores row-banded tile_position matmuls)
            bd_g = [consts.tile([128, NB * 4 * W], bf16, tag=f"bd_g{g}",
                                name=f"bd_g{g}") for g in range(NG)]
            for g in range(NG):
                nc.vector.memset(bd_g[g][:].bitcast(f32), 0.0)
            for g in range(NG):
                nc.sync.dma_start(
                    out=bd_g[g][32 * g:32 * g + 32, :],
                    in_=gt_d[32 * g:32 * g + 32, 0:NB * 4 * W])

            # pred, cast to bf16 during DMA (SWDGE), chunked per (batch,
            # k-group) so each group's matmuls fire as soon as its own
            # columns land; the last chunk gates only one group's tail
            plo_t, phi_t = [], []
            for b in range(NB):
                psrc = pred_d[b].rearrange("k h w -> h k w")
                plo = predp.tile([HL, KW], bf16, tag=f"plo{b}", name=f"plo{b}")
                phi = predp.tile([HL, KW], bf16, tag=f"phi{b}", name=f"phi{b}")
                for k0, k1 in ((0, 4), (4, 8), (8, 12), (12, 16), (16, 17)):
                    nc.gpsimd.dma_start(
                        out=plo[:, k0 * W:k1 * W].rearrange(
                            "p (k w) -> p k w", w=W),
                        in_=psrc[0:HL, k0:k1, :])
                    nc.gpsimd.dma_start(
                        out=phi[:, k0 * W:k1 * W].rearrange(
                            "p (k w) -> p k w", w=W),
                        in_=psrc[HL:H, k0:k1, :])
                plo_t.append(plo)
                phi_t.append(phi)

            negi_t = ctrl[0:96, 0:48].bitcast(bf16)       # [96, 96] -I
            ones_t = ctrl[0:96, 48:49]                    # 1/B
            invd = ctrl[0:NB, 49:50]                      # 1/(sum vis + eps)
            grow0 = [gt[:, b * W:(b + 1) * W] for b in range(NB)]
            grow1 = gt[:, GROW1 - GROW0:GROW1 - GROW0 + W]
            gcol1 = gt[:, GCOL1 - GROW0:GCOL1 - GROW0 + W]
            accall = consts.tile([96, NB], f32, tag="accall")

            # ---------------- main loop ----------------
            accs_t = [gpool.tile([96, NG + 1], f32, tag=f"accs{b}",
                                 name=f"accs{b}") for b in range(NB)]
            for b in range(NB):
                plo = plo_t[b]
                phi = phi_t[b]
                accs = accs_t[b]
                for g in range(NG):
                    vector_group = g == 1 or (g == 2 and b == 3)
                    ps = psump.tile([96, 2048], f32, tag="ps", name="ps")
                    lo = grow0[b][:, 0:HL]
                    hi = grow0[b][:, HL:H]
                    bdt = bd_g[g][:, b * 4 * W:(b + 1) * 4 * W]
                    c0 = g * 4 * W  # start col in (k,w) space for this group
                    # splat targets (pairs of k share one bank); full c=128 so
                    # the HAM clock-gate sees full-array matmuls
                    nc.tensor.matmul(ps[:, 0:384], lo, bdt[:, 0:384],
                                     start=True, stop=vector_group)
                    nc.tensor.matmul(ps[:, 512:896], lo, bdt[:, 384:768],
                                     start=True, stop=vector_group)
                    nc.tensor.matmul(ps[:, 1024:1408], hi, bdt[:, 0:384],
                                     start=True, stop=vector_group)
                    nc.tensor.matmul(ps[:, 1536:1920], hi, bdt[:, 384:768],
                                     start=True, stop=vector_group)
                    if vector_group:
                        # DVE path: psum holds target only; subtract pred on DVE
                        # (one PSUM input allowed), square+reduce from SBUF.
                        diff = scrp.tile([96, 4 * 384], f32, tag="diff", name="diff")
                        psv = ps[:].rearrange("p (a c) -> p a c", c=512)
                        pslo = psv[:, 0:2, 0:384]
                        pshi = psv[:, 2:4, 0:384]
                        dlo = diff[:, 0:768].rearrange("p (a c) -> p a c", c=384)
                        dhi = diff[:, 768:1536].rearrange("p (a c) -> p a c", c=384)
                        plov = plo[:, c0:c0 + 768].rearrange(
                            "p (a c) -> p a c", c=384)
                        phiv = phi[:, c0:c0 + 768].rearrange(
                            "p (a c) -> p a c", c=384)
                        nc.vector.tensor_tensor(dlo, pslo, plov, Alu.subtract)
                        nc.vector.tensor_tensor(dhi, pshi, phiv, Alu.subtract)
                        scr = scrp.tile([96, 4 * 384], f32, tag="scr", name="scr")
                        nc.vector.affine_mul_reduce(
                            out=scr[:], accum_out=accs[:, g:g + 1],
                            in0=diff[:], in1=diff[:], scale=1.0, bias=0.0)
                    else:
                        # accumulate -pred on PE, square+reduce on ScalarE
                        nc.tensor.matmul(ps[:, 0:384], negi_t,
                                         plo[:, c0:c0 + 384],
                                         start=False, stop=True)
                        nc.tensor.matmul(ps[:, 512:896], negi_t,
                                         plo[:, c0 + 384:c0 + 768],
                                         start=False, stop=True)
                        nc.tensor.matmul(ps[:, 1024:1408], negi_t,
                                         phi[:, c0:c0 + 384],
                                         start=False, stop=True)
                        nc.tensor.matmul(ps[:, 1536:1920], negi_t,
                                         phi[:, c0 + 384:c0 + 768],
                                         start=False, stop=True)
                        view = ps[:].rearrange("p (a c) -> p a c", c=512)[:, :, 0:384]
                        scr = scrp.tile([96, 4 * 384], f32, tag="scr", name="scr")
                        sview = scr[:].rearrange("p (a c) -> p a c", c=384)
                        nc.scalar.activation(sview, view, Act.Square,
                                             accum_out=accs[:, g:g + 1])

                # leftover k = 16, then this batch's reduce
                ps = psump.tile([96, 2048], f32, tag="ps", name="ps")
                l1 = grow1[32 * b:32 * b + P, 0:HL]
                h1 = grow1[32 * b:32 * b + P, HL:H]
                gc1 = gcol1[32 * b:32 * b + P, :]
                nc.tensor.matmul(ps[:, 0:192], l1, gc1, start=True, stop=False,
                                 tile_position=(32 * b, 0))
                nc.tensor.matmul(ps[:, 512:704], h1, gc1, start=True, stop=False,
                                 tile_position=(32 * b, 0))
                nc.tensor.matmul(ps[:, 0:192], negi_t, plo[:, 16 * W:17 * W],
                                 start=False, stop=True)
                nc.tensor.matmul(ps[:, 512:704], negi_t, phi[:, 16 * W:17 * W],
                                 start=False, stop=True)
                lview = ps[:].rearrange("p (a c) -> p a c", c=512)[:, 0:2, 0:192]
                scr = scrp.tile([96, 4 * 384], f32, tag="scr", name="scr")
                lsview = scr[:, 0:384].rearrange("p (a c) -> p a c", c=192)
                nc.scalar.activation(lsview, lview, Act.Square,
                                     accum_out=accs[:, NG:NG + 1])

                nc.vector.tensor_reduce(accall[:, b:b + 1], accs[:],
                                        axis=mybir.AxisListType.X, op=Alu.add)

            # ---------------- finalize ----------------
            ps2 = psump.tile([96, 2048], f32, tag="ps", name="ps")
            nc.tensor.matmul(ps2[0:NB, 0:1], accall[:, 0:NB], ones_t,
                             start=True, stop=True)
            outt = consts.tile([NB, 1], f32, tag="outt")
            nc.vector.tensor_tensor(outt[:], ps2[0:NB, 0:1], invd, Alu.mult)
            nc.sync.dma_start(out=out_d[:], in_=outt[:])

    nc.compile()
    return nc


def get_nc():
    if "nc" not in _CACHE:
        _CACHE["nc"] = _build()
    return _CACHE["nc"]


def make_in_maps(pred_heatmaps, keypoints, visibilities):
    import ml_dtypes
    bf = ml_dtypes.bfloat16

    pred = np.ascontiguousarray(pred_heatmaps, dtype=np.float32)
    kp = np.asarray(keypoints, dtype=np.float32)          # [B, P, K, 2]
    vis = np.asarray(visibilities, dtype=np.int32)        # [B, P, K]

    # match the reference's f32 trunc semantics
    x = np.trunc(kp[..., 0] * np.float32(W - 1)).astype(np.float32)  # [B,P,K]
    y = np.trunc(kp[..., 1] * np.float32(H - 1)).astype(np.float32)
    valid = (vis > 0).astype(np.float32)
    hh = np.arange(H, dtype=np.float32)
    grow = np.exp(-((hh[None, None, None, :] - x[..., None]) ** 2) * INV2S2)
    grow *= valid[..., None]                               # [B,P,K,H]
    gcol = np.exp(-((hh[None, None, None, :] - y[..., None]) ** 2) * INV2S2)

    negi = (-np.eye(96, dtype=np.float32)).astype(bf)      # [96, 96]
    den = vis.reshape(B, -1).sum(axis=1).astype(np.float32) + np.float32(1e-6)

    in_maps = []
    for c in range(NCORES):
        bs = slice(c * NB, (c + 1) * NB)
        gw = grow[bs]                                      # [NB,P,K,H]
        gc = gcol[bs]

        gtens = np.zeros((128, GCOLS), dtype=bf)
        # band-packed block-diag gcol: partition 32g+8j+p (k=4g+j),
        # cols (b, j', w) nonzero only at j'==j
        for k in range(16):
            g, j = k // 4, k % 4
            for b in range(NB):
                gtens[8 * k:8 * k + P,
                      b * 4 * W + j * W:b * 4 * W + (j + 1) * W] = \
                    gc[b, :, k, :].astype(bf)
        # grow0: partition 8k+p, cols (b, h)
        for k in range(16):
            for b in range(NB):
                gtens[8 * k:8 * k + P, GROW0 + b * W:GROW0 + (b + 1) * W] = \
                    gw[b, :, k, :].astype(bf)
        # k16 tiles: partition 32b+p
        for b in range(NB):
            gtens[32 * b:32 * b + P, GROW1:GROW1 + W] = gw[b, :, 16, :].astype(bf)
            gtens[32 * b:32 * b + P, GCOL1:GCOL1 + W] = gc[b, :, 16, :].astype(bf)

        ctrl = np.zeros((128, CCOLS), dtype=np.float32)
        # bf16 [96, 96] viewed as raw f32 words [96, 48]
        ctrl[0:96, 0:48] = np.ascontiguousarray(negi).view(np.float32)
        ctrl[0:96, 48] = 1.0 / B
        ctrl[0:NB, 49] = 1.0 / den[bs]

        in_maps.append({
            "pred": pred[bs],
            "gtens": gtens,
            "ctrl": ctrl,
        })
    return in_maps


def kernel(pred_heatmaps, keypoints, visibilities):
    from concourse.bass_utils import run_bass_kernel_spmd

    nc = get_nc()
    in_maps = make_in_maps(pred_heatmaps, keypoints, visibilities)
    res = run_bass_kernel_spmd(nc, in_maps, core_ids=list(range(NCORES)))
    total = np.float64(0.0)
    for c in range(NCORES):
        total += np.asarray(res.results[c]["out"], dtype=np.float64).sum()
    return np.float32(total)
